# revision 1
# baseline (speedup 1.0000x reference)
"""Trainium2 Bass kernel for nn_CrossAttentionExpert.

Problem (hardcoded shapes): B=4, C=256, H=W=64 (N=4096), C8=32.
  cross_p2v = attn(q=wq_p@f_p, k=wk_v@f_v, v=wv_v@f_v)
  cross_v2p = attn(q=wq_v@f_v, k=wk_p@f_p, v=wv_p@f_p)
  out = BN(w_out @ concat([f_p, f_v, cross_p2v, cross_v2p]))  (training-mode BN)

Sharding: 8 cores = (batch b, spatial half h).  Each core computes both
attention directions for its 2048 query positions (keys/values span all
4096 positions of its batch), the fused 1x1 output conv, and BN with a
[128,4] fp32 AllReduce of per-channel sum/sumsq across all 8 cores.

Key layout trick: scores are computed transposed, S^T[n,m] (n=key on
partitions, m=query on free axis) so that the exp'd probabilities can be
used directly as the moving operand of the V^T matmul — no transposes
anywhere on-chip.  All weight transposes are done host-side in numpy.
Softmax skips the max-subtraction (logits are O(25), exp fits fp32 with
huge margin for this problem's 0.05-scaled weights) and the 1/rowsum is
applied after the V-matmul via a PE outer-product broadcast.

All big matmuls run in float32r (PE fast-fp32, 4x throughput at N>=512);
f32r requires dst partition offset 0 and no tile_position, hence the
[32, *] Q/K layouts.  HW pitfalls found by bisect: tensor_tensor_reduce
(dual-output DVE) and activation-with-bias-AP crash the device — use
mul+reduce_sum and tensor_scalar instead.
"""

import numpy as np

import concourse.bass as bass
import concourse.mybir as mybir
import concourse.tile as tile
from concourse import bacc, bass_utils

FP = mybir.dt.float32
FR = mybir.dt.float32r  # PE fast-fp32 mode, 4x matmul throughput at N>=256
P = 128
C = 256
C8 = 32
N = 4096          # full spatial positions per batch
M = 2048          # local query positions per core
NMT = 4           # m-tiles of 512
MT = 512
NCORES = 8
BN_EPS = 1e-5
BN_COUNT = 4 * 4096  # B * H * W

_ALU = mybir.AluOpType
_ACT = mybir.ActivationFunctionType

_PROGRAM = None

# Debug bisect switches (set before first _get_program() call).
DBG_SKIP_ATTN = False       # skip attention (direct conv + BN only)
DBG_SKIP_COLLECTIVE = False  # use local stats instead of AllReduce
DBG_LEVEL = 99  # 1: loads+collective+writeback, 2: +direct conv, 3+: +BN math


def _build_program():
    nc = bacc.Bacc("TRN2", target_bir_lowering=False, debug=False,
                   num_devices=NCORES)

    # ---- DRAM I/O ----
    kv = [nc.dram_tensor(f"kv{d}", [C, N], FR, kind="ExternalInput").ap()
          for d in range(2)]
    wq = [nc.dram_tensor(f"wq{d}", [C, C8], FR, kind="ExternalInput").ap()
          for d in range(2)]
    wk = [nc.dram_tensor(f"wk{d}", [C, C8], FR, kind="ExternalInput").ap()
          for d in range(2)]
    wv = [nc.dram_tensor(f"wv{d}", [C, C], FR, kind="ExternalInput").ap()
          for d in range(2)]
    wout = nc.dram_tensor("wout", [4 * C, C], FR, kind="ExternalInput").ap()
    woutc = nc.dram_tensor("woutc", [4 * C, C], FP, kind="ExternalInput").ap()
    biasq = nc.dram_tensor("biasq", [P, 4], FP, kind="ExternalInput").ap()
    cvec = nc.dram_tensor("cvec", [P, 8], FP, kind="ExternalInput").ap()
    yout = nc.dram_tensor("y", [C, M], FP, kind="ExternalOutput").ap()

    with tile.TileContext(nc) as tc:
        with (
            tc.tile_pool(name="consts", bufs=1) as consts,
            tc.tile_pool(name="big", bufs=1) as big,
            tc.tile_pool(name="vt", bufs=32) as vtp,
            tc.tile_pool(name="st", bufs=1) as stp,
            tc.tile_pool(name="racc", bufs=1) as p_racc,
            tc.tile_pool(name="rp", bufs=1) as p_rp,
            tc.tile_pool(name="rbc", bufs=1) as p_rbc,
            tc.tile_pool(name="cross", bufs=2) as p_cross,
            tc.tile_pool(name="rinvp", bufs=1) as p_rinv,
            tc.tile_pool(name="small", bufs=4) as p_small,
            tc.tile_pool(name="psA", bufs=2, space="PSUM") as psA,
            tc.tile_pool(name="psB", bufs=2, space="PSUM") as psB,
            tc.tile_pool(name="psC", bufs=2, space="PSUM") as psC,
            tc.tile_pool(name="dram", bufs=1, space="DRAM") as dram,
        ):
            # ---- load constants / inputs to SBUF ----
            kv_sb = []
            for d in range(2):
                t = big.tile([P, 2, N], FR, name=f"kvsb{d}")
                src = kv[d].rearrange("(o p) n -> p o n", p=P)
                for o in range(2):
                    for q in range(4):
                        sl = slice(q * 1024, (q + 1) * 1024)
                        nc.sync.dma_start(t[:, o, sl], src[:, o, sl])
                kv_sb.append(t)

            def load_w(ap, shape, name, dt=FR):
                t = consts.tile(shape, dt, name=name)
                nc.sync.dma_start(
                    t[:], ap.rearrange("(o p) m -> p o m", p=P))
                return t

            wq_sb = [load_w(wq[d], [P, 2, C8], f"wqsb{d}") for d in range(2)]
            wk_sb = [load_w(wk[d], [P, 2, C8], f"wksb{d}") for d in range(2)]
            wv_sb = [load_w(wv[d], [P, 2, C], f"wvsb{d}") for d in range(2)]
            wout_sb = load_w(wout, [P, 8, C], "woutsb")
            woutc_sb = load_w(woutc, [P, 8, C], "woutcsb", dt=FP)
            biasq_sb = consts.tile([P, 4], FP, name="biasqsb")
            nc.sync.dma_start(biasq_sb[:], biasq[:])
            cvec_sb = consts.tile([P, 8], FP, name="cvecsb")
            nc.sync.dma_start(cvec_sb[:], cvec[:])

            ones_col = consts.tile([P, 1], FP, name="ones_col")
            nc.vector.memset(ones_col[:], 1.0)
            ones_row = consts.tile([1, P], FP, name="ones_row")
            nc.vector.memset(ones_row[:], 1.0)
            eps_t = consts.tile([P, 1], FP, name="eps_t")
            nc.vector.memset(eps_t[:], BN_EPS)

            # ---- persistent activations ----
            # qr[d]: Q result, C8 channels on partitions 0-31, [32, 2048]
            # kt[d]: K result, C8 on partitions 0-31, keys on free, [32, 4096]
            # (f32r matmuls require dst partition 0 / no tile_position)
            qr = [big.tile([32, M], FR, name=f"qr{d}") for d in range(2)]
            kt = [big.tile([32, N], FR, name=f"kt{d}") for d in range(2)]
            y_acc = [big.tile([P, M], FP, name=f"yacc{cc}") for cc in range(2)]

            # ---- direct terms of the output conv:
            #      y = wout[:, :256] @ f_p[:, half] + wout[:, 256:512] @ f_v[:, half]
            # f_p half = kv1[:, :2048]; f_v half = kv0[:, :2048].
            if DBG_LEVEL < 2:
                for cc in range(2):
                    nc.vector.memset(y_acc[cc][:], 0.5)
            for oc in range(2 if DBG_LEVEL >= 2 else 0):
                ocs = slice(oc * P, (oc + 1) * P)
                for t in range(NMT):
                    msl = slice(t * MT, (t + 1) * MT)
                    ps = psC.tile([P, MT], FP, tag="misc")
                    nc.tensor.matmul(ps, wout_sb[:, 0, ocs],
                                     kv_sb[1][:, 0, msl],
                                     start=True, stop=False)
                    nc.tensor.matmul(ps, wout_sb[:, 1, ocs],
                                     kv_sb[1][:, 1, msl],
                                     start=False, stop=False)
                    nc.tensor.matmul(ps, wout_sb[:, 2, ocs],
                                     kv_sb[0][:, 0, msl],
                                     start=False, stop=False)
                    nc.tensor.matmul(ps, wout_sb[:, 3, ocs],
                                     kv_sb[0][:, 1, msl],
                                     start=False, stop=True)
                    nc.scalar.copy(y_acc[oc][:, msl], ps)

            # ---- per-direction work ----
            for d in range(2 if not DBG_SKIP_ATTN else 0):
                qkv = kv_sb[1 - d]    # Q source (dir0: f_p=kv1, dir1: f_v=kv0)
                kkv = kv_sb[d]        # K/V source

                # Q conv: single [32, M] result, C8 channels on partitions 0-31.
                for t in range(NMT):
                    msl = slice(t * MT, (t + 1) * MT)
                    ps = psC.tile([32, MT], FP, tag="misc")
                    for kc in range(2):
                        nc.tensor.matmul(
                            ps, wq_sb[d][:, kc, :], qkv[:, kc, msl],
                            start=(kc == 0), stop=(kc == 1))
                    nc.scalar.activation(qr[d][:, msl], ps, _ACT.Identity,
                                         bias=biasq_sb[0:32, 2 * d:2 * d + 1])

                # K conv: [32, N], all 4096 keys along the free axis.
                for sub in range(8):
                    nsl = slice(sub * MT, (sub + 1) * MT)
                    ps = psC.tile([32, MT], FP, tag="misc")
                    for kc in range(2):
                        nc.tensor.matmul(
                            ps, wk_sb[d][:, kc, :], kkv[:, kc, nsl],
                            start=(kc == 0), stop=(kc == 1))
                    nc.scalar.activation(
                        kt[d][:, nsl], ps, _ACT.Identity,
                        bias=biasq_sb[0:32, 2 * d + 1:2 * d + 2])

                # V^T conv: vt[j] = f_kv[:, j*128:(j+1)*128]^T @ wv^T, [128, 256]
                vt_d = []
                for j in range(32):
                    ps = psC.tile([P, C], FP, tag="misc")
                    for kc in range(2):
                        nc.tensor.matmul(
                            ps, kkv[:, kc, j * P:(j + 1) * P],
                            wv_sb[d][:, kc, :],
                            start=(kc == 0), stop=(kc == 1))
                    v = vtp.tile([P, C], FR, tag="vt")
                    nc.scalar.copy(v[:], ps)
                    vt_d.append(v)

                # ---- attention over m-tiles ----
                for t in range(NMT):
                    msl = slice(t * MT, (t + 1) * MT)
                    av = [psB.tile([P, MT], FP, tag="av", name=f"av{i}")
                          for i in range(2)]
                    racc = p_racc.tile([P, MT], FP, tag="racc")
                    for burst in range(8):
                        bsl = slice(burst * P, (burst + 1) * P)
                        stg = stp.tile([P, 4 * MT], FR, tag="st")
                        for half in range(2):
                            pt = psA.tile([P, 2, MT], FP, tag="stps")
                            for rr in range(2):
                                rg = 2 * half + rr
                                ksl = slice(rg * 1024 + burst * P,
                                            rg * 1024 + (burst + 1) * P)
                                nc.tensor.matmul(
                                    pt[:, rr, :], kt[d][:, ksl],
                                    qr[d][:, msl],
                                    start=True, stop=True)
                            nc.scalar.activation(
                                stg[:, half * 1024:(half + 1) * 1024],
                                pt[:, :, :], _ACT.Exp)
                        # rowsum partials (sum over the 4 key-chunks here)
                        view = stg[:].rearrange("p (r m) -> p m r", m=MT)
                        if burst == 0:
                            nc.vector.reduce_sum(racc[:], view,
                                                 axis=mybir.AxisListType.X)
                        else:
                            rp = p_rp.tile([P, MT], FP, tag="rp")
                            nc.vector.reduce_sum(rp[:], view,
                                                 axis=mybir.AxisListType.X)
                            nc.vector.tensor_add(racc[:], racc[:], rp[:])
                        # V^T @ P accumulation
                        for rg in range(4):
                            j = rg * 8 + burst
                            ssl = slice(rg * MT, (rg + 1) * MT)
                            for cc in range(2):
                                nc.tensor.matmul(
                                    av[cc], vt_d[j][:, cc * P:(cc + 1) * P],
                                    stg[:, ssl],
                                    start=(burst == 0 and rg == 0),
                                    stop=(burst == 7 and rg == 3))
                    # 1/rowsum, broadcast to 128 partitions via outer product
                    rsum_ps = psC.tile([1, MT], FP, tag="misc")
                    nc.tensor.matmul(rsum_ps, ones_col[:], racc[:],
                                     start=True, stop=True)
                    rinv = p_rinv.tile([1, MT], FP, tag="rinv")
                    nc.vector.reciprocal(rinv[:], rsum_ps)
                    rbc_ps = psC.tile([P, MT], FP, tag="misc")
                    nc.tensor.matmul(rbc_ps, ones_row[:], rinv[:],
                                     start=True, stop=True)
                    rbc = p_rbc.tile([P, MT], FP, tag="rbc")
                    nc.vector.tensor_copy(rbc[:], rbc_ps)
                    # cross = av * (1/rowsum) + bv ; then y += wout_cross @ cross
                    crs = []
                    for cc in range(2):
                        cross = p_cross.tile([P, MT], FP, tag="cross")
                        nc.vector.tensor_mul(cross[:], av[cc], rbc[:])
                        nc.vector.tensor_scalar_add(
                            cross[:], cross[:],
                            cvec_sb[:, 2 * d + cc:2 * d + cc + 1])
                        crs.append(cross)
                    for oc in range(2):
                        ocs = slice(oc * P, (oc + 1) * P)
                        yc = psC.tile([P, MT], FP, tag="misc")
                        nc.tensor.matmul(yc, woutc_sb[:, 4 + 2 * d, ocs],
                                         crs[0][:], start=True, stop=False)
                        nc.tensor.matmul(yc, woutc_sb[:, 5 + 2 * d, ocs],
                                         crs[1][:], start=False, stop=True)
                        nc.vector.tensor_add(y_acc[oc][:, msl],
                                             y_acc[oc][:, msl], yc)

            # ---- BN: local stats, AllReduce, normalize ----
            stats = p_small.tile([P, 4], FP, tag="stats")
            if DBG_LEVEL < 3:
                nc.vector.memset(stats[:], 1.0)
            for cc in range(2 if DBG_LEVEL >= 3 else 0):
                nc.vector.reduce_sum(stats[:, cc:cc + 1], y_acc[cc][:],
                                     axis=mybir.AxisListType.X)
                scratch = stp.tile([P, 4 * MT], FP, tag="st")
                nc.vector.tensor_mul(scratch[:], y_acc[cc][:], y_acc[cc][:])
                nc.vector.reduce_sum(stats[:, 2 + cc:3 + cc], scratch[:],
                                     axis=mybir.AxisListType.X)
            cc_in = dram.tile([P, 4], FP)
            cc_out = dram.tile([P, 4], FP)
            nc.sync.dma_start(cc_in[:], stats[:])
            if DBG_SKIP_COLLECTIVE:
                nc.sync.dma_start(cc_out[:], cc_in[:])
            else:
                nc.gpsimd.collective_compute(
                    "AllReduce", _ALU.add,
                    replica_groups=[list(range(NCORES))],
                    ins=[cc_in.opt()], outs=[cc_out.opt()])
            ar = p_small.tile([P, 4], FP, tag="ar")
            nc.sync.dma_start(ar[:], cc_out[:])

            inv_n = 1.0 / BN_COUNT
            yo = yout.rearrange("(o p) m -> p o m", p=P)
            for cc in range(2):
                if DBG_LEVEL >= 3:
                    mean = p_small.tile([P, 1], FP, tag="bn")
                    ex2 = p_small.tile([P, 1], FP, tag="bn")
                    var = p_small.tile([P, 1], FP, tag="bn")
                    nc.vector.tensor_scalar_mul(mean[:], ar[:, cc:cc + 1],
                                                inv_n)
                    nc.vector.tensor_scalar_mul(ex2[:], ar[:, 2 + cc:3 + cc],
                                                inv_n)
                    nc.vector.tensor_tensor(var[:], mean[:], mean[:],
                                            _ALU.mult)
                    nc.vector.tensor_sub(var[:], ex2[:], var[:])
                    sd = p_small.tile([P, 1], FP, tag="bn")
                    nc.vector.tensor_scalar_add(var[:], var[:], BN_EPS)
                    nc.scalar.activation(sd[:], var[:], _ACT.Sqrt)
                    rstd = p_small.tile([P, 1], FP, tag="bn")
                    nc.vector.reciprocal(rstd[:], sd[:])
                    scale = p_small.tile([P, 1], FP, tag="bn")
                    nc.vector.tensor_tensor(scale[:],
                                            cvec_sb[:, 4 + cc:5 + cc],
                                            rstd[:], _ALU.mult)
                    shift = p_small.tile([P, 1], FP, tag="bn")
                    nc.vector.tensor_tensor(shift[:], mean[:], scale[:],
                                            _ALU.mult)
                    nc.vector.tensor_sub(shift[:], cvec_sb[:, 6 + cc:7 + cc],
                                         shift[:])
                    nc.vector.tensor_scalar(
                        out=y_acc[cc][:], in0=y_acc[cc][:],
                        scalar1=scale[:], scalar2=shift[:],
                        op0=_ALU.mult, op1=_ALU.add)
                for q in range(2):
                    qsl = slice(q * 1024, (q + 1) * 1024)
                    nc.sync.dma_start(yo[:, cc, qsl], y_acc[cc][:, qsl])

    nc.compile()
    return nc


def _get_program():
    global _PROGRAM
    if _PROGRAM is None:
        _PROGRAM = _build_program()
    return _PROGRAM


def _make_in_maps(inputs):
    f_p = np.ascontiguousarray(
        np.asarray(inputs["f_p"], np.float32).reshape(4, C, N))
    f_v = np.ascontiguousarray(
        np.asarray(inputs["f_v"], np.float32).reshape(4, C, N))

    def T(x):
        return np.ascontiguousarray(np.asarray(x, np.float32).T)

    # direction 0 (p2v): q from f_p, k/v from f_v; dir 1 (v2p): reversed.
    shared = {
        "wq0": T(inputs["wq_p"]), "wk0": T(inputs["wk_v"]),
        "wv0": T(inputs["wv_v"]),
        "wq1": T(inputs["wq_v"]), "wk1": T(inputs["wk_p"]),
        "wv1": T(inputs["wv_p"]),
        "wout": T(inputs["w_out"]),
        "woutc": T(inputs["w_out"]),
        "biasq": np.ascontiguousarray(np.stack(
            [np.tile(np.asarray(inputs[k], np.float32), 4)
             for k in ("bq_p", "bk_v", "bq_v", "bk_p")], axis=1)),
        "cvec": np.ascontiguousarray(np.stack(
            [np.asarray(inputs["bv_v"], np.float32)[:P],
             np.asarray(inputs["bv_v"], np.float32)[P:],
             np.asarray(inputs["bv_p"], np.float32)[:P],
             np.asarray(inputs["bv_p"], np.float32)[P:],
             np.asarray(inputs["gamma"], np.float32)[:P],
             np.asarray(inputs["gamma"], np.float32)[P:],
             np.asarray(inputs["beta"], np.float32)[:P],
             np.asarray(inputs["beta"], np.float32)[P:]], axis=1)),
    }
    in_maps = []
    for core in range(NCORES):
        b, h = divmod(core, 2)
        # roll so this core's query half sits at columns [0, 2048); K/V use
        # the full (permuted) range — softmax/AV are order-invariant in keys.
        kv1 = np.ascontiguousarray(np.roll(f_p[b], -h * M, axis=1))
        kv0 = np.ascontiguousarray(np.roll(f_v[b], -h * M, axis=1))
        in_maps.append({"kv0": kv0, "kv1": kv1, **shared})
    return in_maps


def _assemble(results):
    out = np.empty((4, C, N), np.float32)
    for core in range(NCORES):
        b, h = divmod(core, 2)
        out[b][:, h * M:(h + 1) * M] = results[core]["y"]
    return out.reshape(4, C, 64, 64)


def _run(inputs, **kwargs):
    nc = _get_program()
    in_maps = _make_in_maps(inputs)
    res = bass_utils.run_bass_kernel_spmd(
        nc, in_maps, core_ids=list(range(NCORES)), **kwargs)
    return _assemble(res.results), res


def kernel(**inputs):
    out, _ = _run(inputs)
    return out



# revision 2
# speedup vs baseline: 1.9246x; 1.9246x over previous
"""Trainium2 Bass kernel for nn_CrossAttentionExpert (optimized v2).

Problem (hardcoded shapes): B=4, C=256, H=W=64 (N=4096), C8=32.
  cross_p2v = attn(q=wq_p@f_p, k=wk_v@f_v, v=wv_v@f_v)
  cross_v2p = attn(q=wq_v@f_v, k=wk_p@f_p, v=wv_p@f_p)
  out = BN(w_out @ concat([f_p, f_v, cross_p2v, cross_v2p]))  (training BN)

Sharding: 8 cores = (batch b, spatial half h).  Each core computes both
attention directions for its 2048 query positions (keys span all 4096
positions of its batch), plus BN with a [128,4] AllReduce of per-channel
sum/sumsq.

v2 changes vs the f32r baseline (695us):
- All matmuls in bf16 (1 col/cycle streaming, FWL weight loads, less
  power throttle).  Inputs are cast to bf16 host-side (halves DMA too).
- The output conv on the cross terms is folded into the V conv host-side
  (wv' = w_out[:,cross] @ wv), so AV directly produces y-contributions;
  V-bias is dropped entirely (it shifts y by a per-channel constant which
  training-mode BN cancels exactly).
- Scores S^T keep keys-on-partitions (probs feed AV with no transpose);
  the K=32 contraction is packed 4x onto the PE via tile_position row
  tiling, with kt/qr replicated across the 4 partition groups for free by
  replicating the tiny conv weights 4x along the stationary columns.
- Software pipeline over (dir, mtile): AV matmuls of tile t-1 run on the
  PE while ACT exps tile t's scores; PSUM = 4-bank score group + 2-bank
  AV accumulator + 2 misc banks.
- Softmax denominator: contiguous bf16 pairwise adds on DVE (2x rate)
  then gpsimd.partition_all_reduce to sum the 128 key-partitions and
  broadcast; reciprocal+scale applied to the 256ch AV output, not the
  32x-larger prob matrix.
"""

import numpy as np
import ml_dtypes

import concourse.bass as bass
import concourse.bass_isa as bass_isa
import concourse.mybir as mybir
import concourse.tile as tile
from concourse import bacc, bass_utils

FP = mybir.dt.float32
BF = mybir.dt.bfloat16
P = 128
C = 256
C8 = 32
N = 4096          # keys per batch
M = 2048          # local query positions per core
NMT = 4           # m-tiles of 512
MT = 512
NCH = 32          # key chunks of 128 per m-tile
NGR = 8           # groups of 4 key chunks
NCORES = 8
BN_EPS = 1e-5
BN_COUNT = 4 * 4096

_ALU = mybir.AluOpType
_ACT = mybir.ActivationFunctionType

_PROGRAM = None


def _build_program():
    nc = bacc.Bacc("TRN2", target_bir_lowering=False, debug=False,
                   num_devices=NCORES)

    # ---- DRAM I/O ----
    # kv0 = f_v (rolled), kv1 = f_p (rolled), bf16
    kv = [nc.dram_tensor(f"kv{d}", [C, N], BF, kind="ExternalInput").ap()
          for d in range(2)]
    # per-dir replicated q/k conv weights [C, 128] (4 copies of [C,32])
    wq = [nc.dram_tensor(f"wq{d}", [C, P], BF, kind="ExternalInput").ap()
          for d in range(2)]
    wk = [nc.dram_tensor(f"wk{d}", [C, P], BF, kind="ExternalInput").ap()
          for d in range(2)]
    # per-dir fused V conv (w_out[:,cross_d] @ wv_d)^T  [C, C]
    wv = [nc.dram_tensor(f"wv{d}", [C, C], BF, kind="ExternalInput").ap()
          for d in range(2)]
    # direct part of out conv, transposed: [2C, C] (rows: f_p chans then f_v)
    wdir = nc.dram_tensor("wdir", [2 * C, C], BF, kind="ExternalInput").ap()
    # [128, 4] fp32: (bq_d0 x4, bk_d0 x4, bq_d1 x4, bk_d1 x4)
    qkbias = nc.dram_tensor("qkbias", [P, 4], FP, kind="ExternalInput").ap()
    # [128, 4] fp32: gamma_cc0, gamma_cc1, beta_cc0, beta_cc1
    gb = nc.dram_tensor("gb", [P, 4], FP, kind="ExternalInput").ap()
    yout = nc.dram_tensor("y", [C, M], FP, kind="ExternalOutput").ap()

    with tile.TileContext(nc) as tc:
        with (
            tc.tile_pool(name="consts", bufs=1) as consts,
            tc.tile_pool(name="big", bufs=1) as big,
            tc.tile_pool(name="kt", bufs=2) as p_kt,
            tc.tile_pool(name="qr", bufs=2) as p_qr,
            tc.tile_pool(name="vt", bufs=2) as p_vt,
            tc.tile_pool(name="stg", bufs=2) as p_stg,
            tc.tile_pool(name="row", bufs=2) as p_row,
            tc.tile_pool(name="small", bufs=4) as p_small,
            tc.tile_pool(name="ps4", bufs=1, space="PSUM") as ps4p,
            tc.tile_pool(name="psav", bufs=1, space="PSUM") as psavp,
            tc.tile_pool(name="psm", bufs=2, space="PSUM") as psm,
            tc.tile_pool(name="dram", bufs=1, space="DRAM") as dram,
        ):
            # ---- load inputs / constants ----
            kv_sb = []
            for d in range(2):
                t = big.tile([P, 2, N], BF, name=f"kvsb{d}")
                src = kv[d].rearrange("(o p) n -> p o n", p=P)
                for o in range(2):
                    for q in range(4):
                        sl = slice(q * 1024, (q + 1) * 1024)
                        nc.sync.dma_start(t[:, o, sl], src[:, o, sl])
                kv_sb.append(t)

            def load_w(ap, shape, name, dt=BF):
                t = consts.tile(shape, dt, name=name)
                nc.sync.dma_start(t[:], ap.rearrange("(o p) m -> p o m", p=P))
                return t

            wq_sb = [load_w(wq[d], [P, 2, P], f"wqsb{d}") for d in range(2)]
            wk_sb = [load_w(wk[d], [P, 2, P], f"wksb{d}") for d in range(2)]
            wv_sb = [load_w(wv[d], [P, 2, C], f"wvsb{d}") for d in range(2)]
            wdir_sb = load_w(wdir, [P, 4, C], "wdirsb")
            qkb_sb = consts.tile([P, 4], FP, name="qkbsb")
            nc.sync.dma_start(qkb_sb[:], qkbias[:])
            gb_sb = consts.tile([P, 4], FP, name="gbsb")
            nc.sync.dma_start(gb_sb[:], gb[:])

            # persistent output accumulator [oc-half, 2048] fp32
            y_acc = [big.tile([P, M], FP, name=f"yacc{cc}") for cc in range(2)]
            # per-(cc, mtile) BN partial sums / sumsq
            stats_s = big.tile([P, 8], FP, name="stats_s")
            stats_q = big.tile([P, 8], FP, name="stats_q")
            scr = big.tile([P, MT], BF, name="scr")  # discard target

            # ---- direct terms: y = wdir[0:256]^T f_p + wdir[256:512]^T f_v
            for oc in range(2):
                ocs = slice(oc * P, (oc + 1) * P)
                for t in range(NMT):
                    msl = slice(t * MT, (t + 1) * MT)
                    ps = psm.tile([P, MT], FP, tag="misc")
                    nc.tensor.matmul(ps, wdir_sb[:, 0, ocs],
                                     kv_sb[1][:, 0, msl],
                                     start=True, stop=False)
                    nc.tensor.matmul(ps, wdir_sb[:, 1, ocs],
                                     kv_sb[1][:, 1, msl],
                                     start=False, stop=False)
                    nc.tensor.matmul(ps, wdir_sb[:, 2, ocs],
                                     kv_sb[0][:, 0, msl],
                                     start=False, stop=False)
                    nc.tensor.matmul(ps, wdir_sb[:, 3, ocs],
                                     kv_sb[0][:, 1, msl],
                                     start=False, stop=True)
                    nc.vector.tensor_copy(y_acc[oc][:, msl], ps)

            def emit_dir_convs(d):
                """Q/K/V convs for direction d.  Returns (qr, kt, vt)."""
                qkv = kv_sb[1 - d]    # Q source (dir0: f_p, dir1: f_v)
                kkv = kv_sb[d]        # K/V source
                # qr_rep [128 = 4 copies of 32 q-chans, 2048] bf16
                qr = p_qr.tile([P, M], BF, tag="qr", name=f"qr{d}")
                for t in range(NMT):
                    msl = slice(t * MT, (t + 1) * MT)
                    ps = psm.tile([P, MT], FP, tag="misc")
                    for kc in range(2):
                        nc.tensor.matmul(ps, wq_sb[d][:, kc, :],
                                         qkv[:, kc, msl],
                                         start=(kc == 0), stop=(kc == 1))
                    nc.vector.tensor_scalar_add(
                        qr[:, msl], ps, qkb_sb[:, 2 * d:2 * d + 1])
                # kt_rep [128 = 4 copies of 32 k-chans, 4096] bf16
                kt = p_kt.tile([P, N], BF, tag="kt", name=f"kt{d}")
                for sub in range(8):
                    nsl = slice(sub * MT, (sub + 1) * MT)
                    ps = psm.tile([P, MT], FP, tag="misc")
                    for kc in range(2):
                        nc.tensor.matmul(ps, wk_sb[d][:, kc, :],
                                         kkv[:, kc, nsl],
                                         start=(kc == 0), stop=(kc == 1))
                    nc.vector.tensor_scalar_add(
                        kt[:, nsl], ps, qkb_sb[:, 2 * d + 1:2 * d + 2])
                # vt [128 keys, chunk j, 256 fused-out chans] bf16
                vt = p_vt.tile([P, NCH, C], BF, tag="vt", name=f"vt{d}")
                for j2 in range(16):  # pairs of key chunks
                    ps = psm.tile([P, 2, C], FP, tag="misc")
                    for jj in range(2):
                        j = 2 * j2 + jj
                        for kc in range(2):
                            nc.tensor.matmul(
                                ps[:, jj, :],
                                kkv[:, kc, j * P:(j + 1) * P],
                                wv_sb[d][:, kc, :],
                                start=(kc == 0), stop=(kc == 1))
                    nc.vector.tensor_copy(vt[:, 2 * j2:2 * j2 + 2, :], ps)
                return qr, kt, vt

            # ---- software pipeline over (dir, mtile) ----
            tiles = [(d, t) for d in range(2) for t in range(NMT)]
            prev = None  # (d, t, stg, av, rinv, msl)
            dir_state = {}

            def emit_av_group(pv, g):
                """AV matmuls for group g of the previous tile."""
                d_, t_, stg_, av_, _, _ = pv
                vt_ = dir_state[d_][2]
                for i in range(4):
                    ch = 4 * g + i
                    for cc in range(2):
                        nc.tensor.matmul(
                            av_[:, cc, :],
                            vt_[:, ch, cc * P:(cc + 1) * P],
                            stg_[:, ch, :],
                            start=(g == 0 and i == 0 and True),
                            stop=(g == NGR - 1 and i == 3 and True),
                            skip_group_check=True)

            def finish_prev(pv):
                """Scale prev tile's AV output by 1/rowsum into y_acc and,
                for dir1 tiles, fold BN partial stats."""
                d_, t_, _, av_, rinv_, msl_ = pv
                for cc in range(2):
                    tmp = p_small.tile([P, MT], FP, tag="avtmp")
                    nc.vector.tensor_mul(tmp[:], av_[:, cc, :], rinv_[:])
                    nc.vector.tensor_add(y_acc[cc][:, msl_],
                                         y_acc[cc][:, msl_], tmp[:])
                if d_ == 1:
                    for cc in range(2):
                        col = slice(cc * 4 + t_, cc * 4 + t_ + 1)
                        nc.scalar.activation(
                            scr[:], y_acc[cc][:, msl_], _ACT.Square,
                            accum_out=stats_q[:, col])
                        nc.vector.reduce_sum(stats_s[:, col],
                                             y_acc[cc][:, msl_],
                                             axis=mybir.AxisListType.X)

            for (d, t) in tiles:
                if t == 0:
                    dir_state[d] = emit_dir_convs(d)
                qr, kt, vt = dir_state[d]
                msl = slice(t * MT, (t + 1) * MT)
                stg = p_stg.tile([P, NCH, MT], BF, tag="stg")
                av = psavp.tile([P, 2, MT], FP, tag="av")
                racc = p_row.tile([P, MT], FP, tag="racc")
                for g in range(NGR):
                    ps = ps4p.tile([P, 4, MT], FP, tag="ps4")
                    for i in range(4):
                        ch = 4 * g + i
                        nc.tensor.matmul(
                            ps[:, i, :],
                            kt[32 * i:32 * (i + 1), ch * P:(ch + 1) * P],
                            qr[32 * i:32 * (i + 1), msl],
                            start=True, stop=True,
                            tile_position=(32 * i, 0))
                    if prev is not None:
                        emit_av_group(prev, g)
                    nc.scalar.activation(stg[:, 4 * g:4 * g + 4, :], ps[:],
                                         _ACT.Exp)
                    # rowsum partials: contiguous bf16 adds (2x DVE rate)
                    t1 = p_small.tile([P, 2, MT], BF, tag="t1")
                    nc.vector.tensor_add(t1[:], stg[:, 4 * g:4 * g + 2, :],
                                         stg[:, 4 * g + 2:4 * g + 4, :])
                    t2 = p_small.tile([P, MT], BF, tag="t2")
                    nc.vector.tensor_add(t2[:], t1[:, 0, :], t1[:, 1, :])
                    if g == 0:
                        nc.vector.tensor_copy(racc[:], t2[:])
                    else:
                        nc.vector.tensor_add(racc[:], racc[:], t2[:])
                # denominator: sum the 128 key-partitions, broadcast, invert
                rbc = p_row.tile([P, MT], FP, tag="rbc")
                nc.gpsimd.partition_all_reduce(rbc[:], racc[:], P,
                                               bass_isa.ReduceOp.add)
                rinv = p_row.tile([P, MT], FP, tag="rinv")
                nc.vector.reciprocal(rinv[:], rbc[:])
                if prev is not None:
                    finish_prev(prev)
                prev = (d, t, stg, av, rinv, msl)

            # drain: last tile's AV + scale + stats
            for g in range(NGR):
                emit_av_group(prev, g)
            finish_prev(prev)

            # ---- BN: pack stats, AllReduce, normalize ----
            stats = p_small.tile([P, 4], FP, tag="stats")
            for cc in range(2):
                nc.vector.reduce_sum(stats[:, cc:cc + 1],
                                     stats_s[:, 4 * cc:4 * cc + 4],
                                     axis=mybir.AxisListType.X)
                nc.vector.reduce_sum(stats[:, 2 + cc:3 + cc],
                                     stats_q[:, 4 * cc:4 * cc + 4],
                                     axis=mybir.AxisListType.X)
            cc_in = dram.tile([P, 4], FP)
            cc_out = dram.tile([P, 4], FP)
            nc.sync.dma_start(cc_in[:], stats[:])
            nc.gpsimd.collective_compute(
                "AllReduce", _ALU.add,
                replica_groups=[list(range(NCORES))],
                ins=[cc_in.opt()], outs=[cc_out.opt()])
            ar = p_small.tile([P, 4], FP, tag="ar")
            nc.sync.dma_start(ar[:], cc_out[:])

            inv_n = 1.0 / BN_COUNT
            yo = yout.rearrange("(o p) m -> p o m", p=P)
            for cc in range(2):
                mean = p_small.tile([P, 1], FP, tag="bn")
                ex2 = p_small.tile([P, 1], FP, tag="bn")
                var = p_small.tile([P, 1], FP, tag="bn")
                nc.vector.tensor_scalar_mul(mean[:], ar[:, cc:cc + 1], inv_n)
                nc.vector.tensor_scalar_mul(ex2[:], ar[:, 2 + cc:3 + cc],
                                            inv_n)
                nc.vector.tensor_tensor(var[:], mean[:], mean[:], _ALU.mult)
                nc.vector.tensor_sub(var[:], ex2[:], var[:])
                sd = p_small.tile([P, 1], FP, tag="bn")
                nc.vector.tensor_scalar_add(var[:], var[:], BN_EPS)
                nc.scalar.activation(sd[:], var[:], _ACT.Sqrt)
                rstd = p_small.tile([P, 1], FP, tag="bn")
                nc.vector.reciprocal(rstd[:], sd[:])
                scale = p_small.tile([P, 1], FP, tag="bn")
                nc.vector.tensor_tensor(scale[:], gb_sb[:, cc:cc + 1],
                                        rstd[:], _ALU.mult)
                shift = p_small.tile([P, 1], FP, tag="bn")
                nc.vector.tensor_tensor(shift[:], mean[:], scale[:],
                                        _ALU.mult)
                nc.vector.tensor_sub(shift[:], gb_sb[:, 2 + cc:3 + cc],
                                     shift[:])
                nc.vector.tensor_scalar(
                    out=y_acc[cc][:], in0=y_acc[cc][:],
                    scalar1=scale[:], scalar2=shift[:],
                    op0=_ALU.mult, op1=_ALU.add)
                for q in range(2):
                    qsl = slice(q * 1024, (q + 1) * 1024)
                    nc.sync.dma_start(yo[:, cc, qsl], y_acc[cc][:, qsl])

    nc.compile()
    return nc


def _get_program():
    global _PROGRAM
    if _PROGRAM is None:
        _PROGRAM = _build_program()
    return _PROGRAM


def _bf(x):
    return np.ascontiguousarray(np.asarray(x, np.float32)).astype(
        ml_dtypes.bfloat16)


def _make_in_maps(inputs):
    f_p = np.ascontiguousarray(
        np.asarray(inputs["f_p"], np.float32).reshape(4, C, N))
    f_v = np.ascontiguousarray(
        np.asarray(inputs["f_v"], np.float32).reshape(4, C, N))

    w_out = np.asarray(inputs["w_out"], np.float32)

    def rep4(w):  # [32, 256] -> [256, 128] (4 col-copies of w^T)
        return np.tile(np.asarray(w, np.float32).T, (1, 4))

    def fused_v(dcol, wv_):  # (w_out[:, dcol] @ wv)^T [256, 256]
        blk = w_out[:, dcol * C:(dcol + 1) * C]
        return (blk @ np.asarray(wv_, np.float32)).T

    def tile4(b):  # [32] -> [128]
        return np.tile(np.asarray(b, np.float32), 4)

    shared = {
        # dir0 (p2v): q from f_p, k/v from f_v; dir1 (v2p): reversed
        "wq0": _bf(rep4(inputs["wq_p"])), "wk0": _bf(rep4(inputs["wk_v"])),
        "wv0": _bf(fused_v(2, inputs["wv_v"])),
        "wq1": _bf(rep4(inputs["wq_v"])), "wk1": _bf(rep4(inputs["wk_p"])),
        "wv1": _bf(fused_v(3, inputs["wv_p"])),
        "wdir": _bf(w_out[:, :2 * C].T),
        "qkbias": np.ascontiguousarray(np.stack(
            [tile4(inputs["bq_p"]), tile4(inputs["bk_v"]),
             tile4(inputs["bq_v"]), tile4(inputs["bk_p"])], axis=1)),
        "gb": np.ascontiguousarray(np.stack(
            [np.asarray(inputs["gamma"], np.float32)[:P],
             np.asarray(inputs["gamma"], np.float32)[P:],
             np.asarray(inputs["beta"], np.float32)[:P],
             np.asarray(inputs["beta"], np.float32)[P:]], axis=1)),
    }
    in_maps = []
    for core in range(NCORES):
        b, h = divmod(core, 2)
        # roll so this core's query half sits at columns [0, 2048); K/V use
        # the full (permuted) range — softmax/AV are order-invariant in keys.
        kv1 = _bf(np.roll(f_p[b], -h * M, axis=1))
        kv0 = _bf(np.roll(f_v[b], -h * M, axis=1))
        in_maps.append({"kv0": kv0, "kv1": kv1, **shared})
    return in_maps


def _assemble(results):
    out = np.empty((4, C, N), np.float32)
    for core in range(NCORES):
        b, h = divmod(core, 2)
        out[b][:, h * M:(h + 1) * M] = results[core]["y"]
    return out.reshape(4, C, 64, 64)


def _run(inputs, **kwargs):
    nc = _get_program()
    in_maps = _make_in_maps(inputs)
    res = bass_utils.run_bass_kernel_spmd(
        nc, in_maps, core_ids=list(range(NCORES)), **kwargs)
    return _assemble(res.results), res


def kernel(**inputs):
    out, _ = _run(inputs)
    return out


# revision 3
# speedup vs baseline: 1.9439x; 1.0100x over previous
"""Trainium2 Bass kernel for nn_CrossAttentionExpert (optimized v3).

Problem (hardcoded shapes): B=4, C=256, H=W=64 (N=4096), C8=32.
  cross_p2v = attn(q=wq_p@f_p, k=wk_v@f_v, v=wv_v@f_v)
  cross_v2p = attn(q=wq_v@f_v, k=wk_p@f_p, v=wv_p@f_p)
  out = BN(w_out @ concat([f_p, f_v, cross_p2v, cross_v2p]))  (training BN)

Sharding: 8 cores = (batch b, spatial half h).  Each core computes both
attention directions for its 2048 query positions (keys span all 4096
positions of its batch), plus BN with a [128,4] AllReduce of per-channel
sum/sumsq.

Design (see git history for the f32r baseline at 695us, v2 at 361us):
- All matmuls bf16 (1 col/cycle, FWL weight loads, less power throttle);
  inputs cast to bf16 host-side (halves DMA).
- Cross-term output conv folded into V host-side (wv' = w_out[:,cross]@wv)
  so AV directly produces y contributions; V-bias dropped entirely (it
  shifts y by a per-channel constant which training-mode BN cancels
  exactly); 1/rowsum applied to the folded 256-ch AV output.
- Scores S^T (keys on partitions feed AV with no transposes); the K=32
  contraction packed 4x via tile_position row tiling, with kt/qr
  replicated across partition groups for free by replicating the tiny
  conv weights 4x along stationary columns (the 4 concurrent matmuls
  share one moving-operand stream).
- Two-deep software pipeline over (dir, mtile): ACT exps tile t while the
  PE runs tile t-1's AV matmuls + spread-out conv "filler" work; PSUM =
  4-bank score group + 2-bank AV accumulator + 2 misc banks.
- Softmax denominator: contiguous bf16 pairwise adds (DVE) +
  gpsimd.partition_all_reduce (sum over key-partitions + broadcast) +
  reciprocal_approx_fast.  The av-scale of tile t-1 is emitted *before*
  tile t's denominator chain so the AV PSUM banks free without stalling
  the PE at mtile boundaries.
"""

import numpy as np
import ml_dtypes

import concourse.bass as bass
import concourse.bass_isa as bass_isa
import concourse.mybir as mybir
import concourse.tile as tile
from concourse import bacc, bass_utils

FP = mybir.dt.float32
BF = mybir.dt.bfloat16
P = 128
C = 256
C8 = 32
N = 4096          # keys per batch
M = 2048          # local query positions per core
NMT = 4           # m-tiles of 512
MT = 512
NCH = 32          # key chunks of 128 per m-tile
NGR = 8           # groups of 4 key chunks
NCORES = 8
BN_EPS = 1e-5
BN_COUNT = 4 * 4096

_ALU = mybir.AluOpType
_ACT = mybir.ActivationFunctionType

_PROGRAM = None


def _build_program():
    nc = bacc.Bacc("TRN2", target_bir_lowering=False, debug=False,
                   num_devices=NCORES)

    # ---- DRAM I/O ----
    # kv0 = f_v (rolled), kv1 = f_p (rolled), bf16
    kv = [nc.dram_tensor(f"kv{d}", [C, N], BF, kind="ExternalInput").ap()
          for d in range(2)]
    wq = [nc.dram_tensor(f"wq{d}", [C, P], BF, kind="ExternalInput").ap()
          for d in range(2)]
    wk = [nc.dram_tensor(f"wk{d}", [C, P], BF, kind="ExternalInput").ap()
          for d in range(2)]
    wv = [nc.dram_tensor(f"wv{d}", [C, C], BF, kind="ExternalInput").ap()
          for d in range(2)]
    wdir = nc.dram_tensor("wdir", [2 * C, C], BF, kind="ExternalInput").ap()
    qkbias = nc.dram_tensor("qkbias", [P, 4], FP, kind="ExternalInput").ap()
    gb = nc.dram_tensor("gb", [P, 4], FP, kind="ExternalInput").ap()
    yout = nc.dram_tensor("y", [C, M], FP, kind="ExternalOutput").ap()

    with tile.TileContext(nc) as tc:
        with (
            tc.tile_pool(name="consts", bufs=1) as consts,
            tc.tile_pool(name="big", bufs=1) as big,
            tc.tile_pool(name="kt", bufs=2) as p_kt,
            tc.tile_pool(name="qr", bufs=2) as p_qr,
            tc.tile_pool(name="vt", bufs=2) as p_vt,
            tc.tile_pool(name="stg", bufs=2) as p_stg,
            tc.tile_pool(name="row", bufs=2) as p_row,
            tc.tile_pool(name="small", bufs=4) as p_small,
            tc.tile_pool(name="ps4", bufs=1, space="PSUM") as ps4p,
            tc.tile_pool(name="psav", bufs=1, space="PSUM") as psavp,
            tc.tile_pool(name="psm", bufs=2, space="PSUM") as psm,
            tc.tile_pool(name="dram", bufs=1, space="DRAM") as dram,
        ):
            # ---- load inputs / constants ----
            kv_sb = []
            for d in range(2):
                t = big.tile([P, 2, N], BF, name=f"kvsb{d}")
                src = kv[d].rearrange("(o p) n -> p o n", p=P)
                for o in range(2):
                    for q in range(4):
                        sl = slice(q * 1024, (q + 1) * 1024)
                        nc.sync.dma_start(t[:, o, sl], src[:, o, sl])
                kv_sb.append(t)

            def load_w(ap, shape, name, dt=BF):
                t = consts.tile(shape, dt, name=name)
                nc.sync.dma_start(t[:], ap.rearrange("(o p) m -> p o m", p=P))
                return t

            wq_sb = [load_w(wq[d], [P, 2, P], f"wqsb{d}") for d in range(2)]
            wk_sb = [load_w(wk[d], [P, 2, P], f"wksb{d}") for d in range(2)]
            wv_sb = [load_w(wv[d], [P, 2, C], f"wvsb{d}") for d in range(2)]
            wdir_sb = load_w(wdir, [P, 4, C], "wdirsb")
            qkb_sb = consts.tile([P, 4], FP, name="qkbsb")
            nc.sync.dma_start(qkb_sb[:], qkbias[:])
            gb_sb = consts.tile([P, 4], FP, name="gbsb")
            nc.sync.dma_start(gb_sb[:], gb[:])

            y_acc = [big.tile([P, M], FP, name=f"yacc{cc}") for cc in range(2)]
            stats_s = big.tile([P, 8], FP, name="stats_s")
            stats_q = big.tile([P, 8], FP, name="stats_q")
            scr = big.tile([P, MT], BF, name="scr")  # discard target

            # ---- per-dir persistent tiles (allocated up front; pools give
            # ---- each dir its own buffer) ----
            qr_t = [p_qr.tile([P, M], BF, tag="qr", name=f"qr{d}")
                    for d in range(2)]
            kt_t = [p_kt.tile([P, N], BF, tag="kt", name=f"kt{d}")
                    for d in range(2)]
            vt_t = [p_vt.tile([P, NCH, C], BF, tag="vt", name=f"vt{d}")
                    for d in range(2)]

            # ---- conv work units (each: a few matmuls + one DVE op) ----
            def unit_direct(oc, t):
                def emit():
                    msl = slice(t * MT, (t + 1) * MT)
                    ocs = slice(oc * P, (oc + 1) * P)
                    ps = psm.tile([P, MT], FP, tag="misc")
                    for j, (kvi, o) in enumerate(
                            ((1, 0), (1, 1), (0, 0), (0, 1))):
                        nc.tensor.matmul(ps, wdir_sb[:, j, ocs],
                                         kv_sb[kvi][:, o, slice(t * MT,
                                                                (t + 1) * MT)],
                                         start=(j == 0), stop=(j == 3))
                    nc.vector.tensor_copy(y_acc[oc][:, msl], ps)
                return emit

            def unit_qr(d, t):
                def emit():
                    qkv = kv_sb[1 - d]
                    msl = slice(t * MT, (t + 1) * MT)
                    ps = psm.tile([P, MT], FP, tag="misc")
                    for kc in range(2):
                        nc.tensor.matmul(ps, wq_sb[d][:, kc, :],
                                         qkv[:, kc, msl],
                                         start=(kc == 0), stop=(kc == 1))
                    nc.vector.tensor_scalar_add(
                        qr_t[d][:, msl], ps, qkb_sb[:, 2 * d:2 * d + 1])
                return emit

            def unit_kt(d, sub):
                def emit():
                    kkv = kv_sb[d]
                    nsl = slice(sub * MT, (sub + 1) * MT)
                    ps = psm.tile([P, MT], FP, tag="misc")
                    for kc in range(2):
                        nc.tensor.matmul(ps, wk_sb[d][:, kc, :],
                                         kkv[:, kc, nsl],
                                         start=(kc == 0), stop=(kc == 1))
                    nc.vector.tensor_scalar_add(
                        kt_t[d][:, nsl], ps, qkb_sb[:, 2 * d + 1:2 * d + 2])
                return emit

            def unit_vt(d, j2):
                def emit():
                    kkv = kv_sb[d]
                    ps = psm.tile([P, 2, C], FP, tag="misc")
                    for jj in range(2):
                        j = 2 * j2 + jj
                        for kc in range(2):
                            nc.tensor.matmul(
                                ps[:, jj, :],
                                kkv[:, kc, j * P:(j + 1) * P],
                                wv_sb[d][:, kc, :],
                                start=(kc == 0), stop=(kc == 1))
                    nc.vector.tensor_copy(vt_t[d][:, 2 * j2:2 * j2 + 2, :],
                                          ps)
                return emit

            # filler schedule: tile index i = 4*d + t -> slot -> units
            fillers = {i: [[] for _ in range(NGR)] for i in range(8)}

            def spread(units, i):
                nslots = NGR
                for u, fn in enumerate(units):
                    fillers[i][(u * nslots) // len(units)].append(fn)

            spread([unit_direct(oc, t) for oc in range(2)
                    for t in range(NMT)] +
                   [unit_vt(0, j2) for j2 in range(16)], 0)
            spread([unit_qr(1, t) for t in range(NMT)] +
                   [unit_kt(1, sub) for sub in range(8)], 2)
            spread([unit_vt(1, j2) for j2 in range(8)], 3)
            spread([unit_vt(1, j2) for j2 in range(8, 16)], 4)

            # prologue: dir0 q/k convs only
            for t in range(NMT):
                unit_qr(0, t)()
            for sub in range(8):
                unit_kt(0, sub)()

            # ---- software pipeline over (dir, mtile) ----
            tiles = [(d, t) for d in range(2) for t in range(NMT)]
            prev = None  # (d, t, stg, av, rinv, msl)

            def emit_av_group(pv, g):
                d_, t_, stg_, av_, _, _ = pv
                for i in range(4):
                    ch = 4 * g + i
                    for cc in range(2):
                        nc.tensor.matmul(
                            av_[:, cc, :],
                            vt_t[d_][:, ch, cc * P:(cc + 1) * P],
                            stg_[:, ch, :],
                            start=(g == 0 and i == 0),
                            stop=(g == NGR - 1 and i == 3),
                            skip_group_check=True)

            def finish_prev(pv):
                """Scale prev tile's AV output by 1/rowsum into y_acc and,
                for dir1 tiles, fold BN partial stats."""
                d_, t_, _, av_, rinv_, msl_ = pv
                for cc in range(2):
                    tmp = p_small.tile([P, MT], FP, tag="avtmp")
                    nc.vector.tensor_mul(tmp[:], av_[:, cc, :], rinv_[:])
                    nc.vector.tensor_add(y_acc[cc][:, msl_],
                                         y_acc[cc][:, msl_], tmp[:])
                if d_ == 1:
                    for cc in range(2):
                        col = slice(cc * 4 + t_, cc * 4 + t_ + 1)
                        nc.scalar.activation(
                            scr[:], y_acc[cc][:, msl_], _ACT.Square,
                            accum_out=stats_q[:, col])
                        nc.vector.reduce_sum(stats_s[:, col],
                                             y_acc[cc][:, msl_],
                                             axis=mybir.AxisListType.X)

            for (d, t) in tiles:
                qr, kt, vt = qr_t[d], kt_t[d], vt_t[d]
                i = 4 * d + t
                msl = slice(t * MT, (t + 1) * MT)
                stg = p_stg.tile([P, NCH, MT], BF, tag="stg")
                av = psavp.tile([P, 2, MT], FP, tag="av")
                racc = p_row.tile([P, MT], FP, tag="racc")
                for g in range(NGR):
                    ps = ps4p.tile([P, 4, MT], FP, tag="ps4")
                    for q in range(4):
                        ch = 4 * g + q
                        nc.tensor.matmul(
                            ps[:, q, :],
                            kt[32 * q:32 * (q + 1), ch * P:(ch + 1) * P],
                            qr[32 * q:32 * (q + 1), msl],
                            start=True, stop=True,
                            tile_position=(32 * q, 0))
                    if prev is not None:
                        emit_av_group(prev, g)
                    for fn in fillers[i][g]:
                        fn()
                    nc.scalar.activation(stg[:, 4 * g:4 * g + 4, :], ps[:],
                                         _ACT.Exp)
                    if g == NGR - 1 and prev is not None:
                        # free prev's AV banks before this tile's denominator
                        # chain so the next tile's AV matmuls aren't stalled
                        finish_prev(prev)
                    # rowsum partials: contiguous bf16 adds (2x DVE rate)
                    t1 = p_small.tile([P, 2, MT], BF, tag="t1")
                    nc.vector.tensor_add(t1[:], stg[:, 4 * g:4 * g + 2, :],
                                         stg[:, 4 * g + 2:4 * g + 4, :])
                    t2 = p_small.tile([P, MT], BF, tag="t2")
                    nc.vector.tensor_add(t2[:], t1[:, 0, :], t1[:, 1, :])
                    if g == 0:
                        nc.vector.tensor_copy(racc[:], t2[:])
                    else:
                        nc.vector.tensor_add(racc[:], racc[:], t2[:])
                rbc = p_row.tile([P, MT], FP, tag="rbc")
                nc.gpsimd.partition_all_reduce(rbc[:], racc[:], P,
                                               bass_isa.ReduceOp.add)
                rinv = p_row.tile([P, MT], FP, tag="rinv")
                nc.vector.reciprocal_approx_fast(out=rinv[:], in_=rbc[:])
                prev = (d, t, stg, av, rinv, msl)

            # drain: last tile's AV + scale + stats
            for g in range(NGR):
                emit_av_group(prev, g)
            finish_prev(prev)

            # ---- BN: pack stats, AllReduce, normalize ----
            stats = p_small.tile([P, 4], FP, tag="stats")
            for cc in range(2):
                nc.vector.reduce_sum(stats[:, cc:cc + 1],
                                     stats_s[:, 4 * cc:4 * cc + 4],
                                     axis=mybir.AxisListType.X)
                nc.vector.reduce_sum(stats[:, 2 + cc:3 + cc],
                                     stats_q[:, 4 * cc:4 * cc + 4],
                                     axis=mybir.AxisListType.X)
            cc_in = dram.tile([P, 4], FP)
            cc_out = dram.tile([P, 4], FP)
            nc.sync.dma_start(cc_in[:], stats[:])
            nc.gpsimd.collective_compute(
                "AllReduce", _ALU.add,
                replica_groups=[list(range(NCORES))],
                ins=[cc_in.opt()], outs=[cc_out.opt()])
            ar = p_small.tile([P, 4], FP, tag="ar")
            nc.sync.dma_start(ar[:], cc_out[:])

            inv_n = 1.0 / BN_COUNT
            yo = yout.rearrange("(o p) m -> p o m", p=P)
            for cc in range(2):
                mean = p_small.tile([P, 1], FP, tag="bn")
                ex2 = p_small.tile([P, 1], FP, tag="bn")
                var = p_small.tile([P, 1], FP, tag="bn")
                nc.vector.tensor_scalar_mul(mean[:], ar[:, cc:cc + 1], inv_n)
                nc.vector.tensor_scalar_mul(ex2[:], ar[:, 2 + cc:3 + cc],
                                            inv_n)
                nc.vector.tensor_tensor(var[:], mean[:], mean[:], _ALU.mult)
                nc.vector.tensor_sub(var[:], ex2[:], var[:])
                sd = p_small.tile([P, 1], FP, tag="bn")
                nc.vector.tensor_scalar_add(var[:], var[:], BN_EPS)
                nc.scalar.activation(sd[:], var[:], _ACT.Sqrt)
                rstd = p_small.tile([P, 1], FP, tag="bn")
                nc.vector.reciprocal(rstd[:], sd[:])
                scale = p_small.tile([P, 1], FP, tag="bn")
                nc.vector.tensor_tensor(scale[:], gb_sb[:, cc:cc + 1],
                                        rstd[:], _ALU.mult)
                shift = p_small.tile([P, 1], FP, tag="bn")
                nc.vector.tensor_tensor(shift[:], mean[:], scale[:],
                                        _ALU.mult)
                nc.vector.tensor_sub(shift[:], gb_sb[:, 2 + cc:3 + cc],
                                     shift[:])
                for q in range(2):
                    qsl = slice(q * 1024, (q + 1) * 1024)
                    nc.vector.tensor_scalar(
                        out=y_acc[cc][:, qsl], in0=y_acc[cc][:, qsl],
                        scalar1=scale[:], scalar2=shift[:],
                        op0=_ALU.mult, op1=_ALU.add)
                    nc.sync.dma_start(yo[:, cc, qsl], y_acc[cc][:, qsl])

    nc.compile()
    return nc


def _get_program():
    global _PROGRAM
    if _PROGRAM is None:
        _PROGRAM = _build_program()
    return _PROGRAM


def _bf(x):
    return np.ascontiguousarray(np.asarray(x, np.float32)).astype(
        ml_dtypes.bfloat16)


def _make_in_maps(inputs):
    f_p = np.ascontiguousarray(
        np.asarray(inputs["f_p"], np.float32).reshape(4, C, N))
    f_v = np.ascontiguousarray(
        np.asarray(inputs["f_v"], np.float32).reshape(4, C, N))

    w_out = np.asarray(inputs["w_out"], np.float32)

    def rep4(w):  # [32, 256] -> [256, 128] (4 col-copies of w^T)
        return np.tile(np.asarray(w, np.float32).T, (1, 4))

    def fused_v(dcol, wv_):  # (w_out[:, dcol] @ wv)^T [256, 256]
        blk = w_out[:, dcol * C:(dcol + 1) * C]
        return (blk @ np.asarray(wv_, np.float32)).T

    def tile4(b):  # [32] -> [128]
        return np.tile(np.asarray(b, np.float32), 4)

    shared = {
        # dir0 (p2v): q from f_p, k/v from f_v; dir1 (v2p): reversed
        "wq0": _bf(rep4(inputs["wq_p"])), "wk0": _bf(rep4(inputs["wk_v"])),
        "wv0": _bf(fused_v(2, inputs["wv_v"])),
        "wq1": _bf(rep4(inputs["wq_v"])), "wk1": _bf(rep4(inputs["wk_p"])),
        "wv1": _bf(fused_v(3, inputs["wv_p"])),
        "wdir": _bf(w_out[:, :2 * C].T),
        "qkbias": np.ascontiguousarray(np.stack(
            [tile4(inputs["bq_p"]), tile4(inputs["bk_v"]),
             tile4(inputs["bq_v"]), tile4(inputs["bk_p"])], axis=1)),
        "gb": np.ascontiguousarray(np.stack(
            [np.asarray(inputs["gamma"], np.float32)[:P],
             np.asarray(inputs["gamma"], np.float32)[P:],
             np.asarray(inputs["beta"], np.float32)[:P],
             np.asarray(inputs["beta"], np.float32)[P:]], axis=1)),
    }
    in_maps = []
    for core in range(NCORES):
        b, h = divmod(core, 2)
        # roll so this core's query half sits at columns [0, 2048); K/V use
        # the full (permuted) range — softmax/AV are order-invariant in keys.
        kv1 = _bf(np.roll(f_p[b], -h * M, axis=1))
        kv0 = _bf(np.roll(f_v[b], -h * M, axis=1))
        in_maps.append({"kv0": kv0, "kv1": kv1, **shared})
    return in_maps


def _assemble(results):
    out = np.empty((4, C, N), np.float32)
    for core in range(NCORES):
        b, h = divmod(core, 2)
        out[b][:, h * M:(h + 1) * M] = results[core]["y"]
    return out.reshape(4, C, 64, 64)


def _run(inputs, **kwargs):
    nc = _get_program()
    in_maps = _make_in_maps(inputs)
    res = bass_utils.run_bass_kernel_spmd(
        nc, in_maps, core_ids=list(range(NCORES)), **kwargs)
    return _assemble(res.results), res


def kernel(**inputs):
    out, _ = _run(inputs)
    return out
